# revision 1
# baseline (speedup 1.0000x reference)
"""Single-head causal attention (B=8, T=2048, D=1024, H=128) on 8 TRN2
NeuronCores — data-parallel over batch (one batch element per core).

Per-core dataflow (bf16 matmul compute, f32 accumulation):
  0. Warmup matmul stream during the x DMA window brings the PE HAM
     clock-gate to 2.4 GHz before the real work starts.
  1. x [T, D] DMA'd naturally (sync + scalar HW queues alternating),
     transposed on TensorE in f32 (128x128 tiles vs identity) straight
     from the f32 tiles — the front is DMA-arrival-bound, so the heavier
     f32 transposes fill the arrival window and keep HAM warm; PSUM->SBUF
     copies cast to bf16 into xT [d-part, d-tile, t].
  2. Weights w[p, dt, h] = W[dt*128+p, h] loaded on the scalar HW queue
     ahead of x. Projections qT/kT/vT in N=512 chunks (weights
     stationary). v_aug [t-tile, 129] (v natural + a ones column that
     makes PV also produce the softmax denominator) built by
     PE-transposing vT tiles.
  3. Scores TRANSPOSED, two k-tiles per PSUM tile: ST[k 128, q 1024] =
     kT_tile^T @ qT_chunk (two matmuls). Paired exp(scale*ST) on ScalarE
     writes PT bf16 — already in the lhsT orientation PV needs (no P
     transposes). Causality: lower-left tiles skipped, diagonal tiles
     compute/exp only the valid column range and zero the 128x128
     triangle via GpSimd affine_select on PT.
  4. O[q 128, 129] += PT_slice^T @ v_aug_tile accumulated over k tiles in
     PSUM; col 128 is the softmax denominator. Divide on DVE, DMA out.
"""

import numpy as np

import concourse.bass as bass
import concourse.bacc as bacc
import concourse.mybir as mybir
import concourse.tile as tile
from concourse import bass_utils
from concourse.masks import make_identity

B, T, D, H = 8, 2048, 1024, 128
P = 128
DT = D // P  # 8 d tiles
TT = T // P  # 16 t tiles
CH = 512  # q chunk width
QC = T // CH  # 4 q chunks
N_CORES = 8
SCALE = float(1.0 / np.sqrt(H))
N_WARMUP = 28

F32 = mybir.dt.float32
BF16 = mybir.dt.bfloat16


def build_nc():
    nc = bacc.Bacc("TRN2", target_bir_lowering=False, debug=False)
    x = nc.dram_tensor("x", [T, D], F32, kind="ExternalInput").ap()
    wq_d = nc.dram_tensor("wq", [D, H], F32, kind="ExternalInput").ap()
    wk_d = nc.dram_tensor("wk", [D, H], F32, kind="ExternalInput").ap()
    wv_d = nc.dram_tensor("wv", [D, H], F32, kind="ExternalInput").ap()
    out = nc.dram_tensor("out", [T, H], F32, kind="ExternalOutput").ap()

    with tile.TileContext(nc) as tc:
        _build_body(nc, tc, x, wq_d, wk_d, wv_d, out)
    nc.compile()
    return nc


def _build_body(nc, tc, x, wq_d, wk_d, wv_d, out):
    with (
        tc.tile_pool(name="persist", bufs=1) as persist,
        tc.tile_pool(name="work", bufs=3) as work,
        tc.tile_pool(name="ps", bufs=1, space="PSUM") as ps,
    ):
        # ---- constants ----
        ident_f = persist.tile([P, P], F32, tag="ident_f", name="ident_f")
        make_identity(nc, ident_f)
        ident_b = persist.tile([P, P], BF16, tag="ident_b", name="ident_b")
        make_identity(nc, ident_b)

        # ---- persistent activations ----
        xT = persist.tile([P, DT, T], BF16, tag="xT", name="xT")
        qT = persist.tile([P, T], BF16, tag="qT", name="qT")
        kT = persist.tile([P, T], BF16, tag="kT", name="kT")
        vT = persist.tile([P, T], BF16, tag="vT", name="vT")
        v_aug = persist.tile([P, TT, H + 1], BF16, tag="v_aug", name="v_aug")
        nc.gpsimd.memset(v_aug[:], 1.0)  # col H stays 1.0 (ones trick)

        # ---- PE warmup during the DMA window (HAM -> 2.4GHz) ----
        warm_ps = ps.tile([P, 3 * (H + 1)], F32, tag="o", bufs=4, name="warm_ps")
        for w in range(N_WARMUP):
            nc.tensor.matmul(
                warm_ps[:], ident_b[:], v_aug[:, 0:3, :], start=True, stop=True
            )

        # ---- weights: [D, H] -> [p, dt, h] (scalar HW queue), cast bf16 ----
        w_bf = []
        for nm, wd in (("wq", wq_d), ("wk", wk_d), ("wv", wv_d)):
            wf = work.tile([P, DT, H], F32, tag="wf32", name=f"{nm}_f32")
            nc.scalar.dma_start(wf[:], wd.rearrange("(a p) h -> p a h", p=P))
            wb = persist.tile([P, DT, H], BF16, tag=f"{nm}_bf", name=f"{nm}_bf")
            nc.vector.tensor_copy(wb[:], wf[:])
            w_bf.append(wb)
        wq_bf, wk_bf, wv_bf = w_bf

        # ---- phase 1: load x, bf16-truncation transposes, projections in
        # half-chunk (N=256) pieces emitted right after each tile pair so the
        # PE stream density matches the DMA arrival cadence (keeps HAM warm
        # without the heavy f32 transposes) ----
        HC = CH // 2  # 256-column projection pieces
        for c in range(QC):
            for hh in range(2):
                for tt in (4 * c + 2 * hh, 4 * c + 2 * hh + 1):
                    x_nat = work.tile(
                        [P, D], F32, tag="x_nat", bufs=4, name=f"x_nat{tt}"
                    )
                    ldeng = nc.sync if tt % 2 == 0 else nc.scalar
                    ldeng.dma_start(x_nat[:], x[tt * P : (tt + 1) * P, :])
                    # bf16 truncation view (top 2 bytes of each LE f32)
                    xv = x_nat.bitcast(BF16)
                    for half in range(2):
                        tr_ps = ps.tile(
                            [P, 4 * P], BF16, tag="mm", bufs=2, name=f"tr{tt}_{half}"
                        )
                        for j in range(4):
                            dt = half * 4 + j
                            nc.tensor.transpose(
                                tr_ps[:, j * P : (j + 1) * P],
                                xv[:, 2 * dt * P + 1 : 2 * (dt + 1) * P : 2],
                                ident_b,
                            )
                        dst = xT[:, half * 4 : half * 4 + 4, tt * P : (tt + 1) * P]
                        src = tr_ps.rearrange("p (a t) -> p a t", a=4)
                        if (tt + half) % 2 == 0:
                            nc.vector.tensor_copy(dst, src)
                        else:
                            nc.scalar.copy(dst, src)
                # qT / kT / vT piece over this tile pair's 256 columns
                t0 = c * CH + hh * HC
                for nm, wb, dstT in (
                    ("q", wq_bf, qT),
                    ("k", wk_bf, kT),
                    ("v", wv_bf, vT),
                ):
                    pr_ps = ps.tile(
                        [P, HC], F32, tag="o", bufs=4, name=f"{nm}T_ps{c}_{hh}"
                    )
                    for dt in range(DT):
                        nc.tensor.matmul(
                            pr_ps[:],
                            wb[:, dt, :],
                            xT[:, dt, t0 : t0 + HC],
                            start=(dt == 0),
                            stop=(dt == DT - 1),
                        )
                    nc.vector.tensor_copy(dstT[:, t0 : t0 + HC], pr_ps[:])
            # v natural tiles for this chunk: PE-transpose vT tiles into v_aug
            for tt in range(4 * c, 4 * c + 4):
                vtr = ps.tile([P, P], BF16, tag="o", bufs=4, name=f"vtr{tt}")
                nc.tensor.transpose(vtr[:], vT[:, tt * P : (tt + 1) * P], ident_b)
                nc.vector.tensor_copy(v_aug[:, tt, 0:H], vtr[:])

        # ---- phase 2: attention main loop (k-tiles processed in pairs) ----
        for c in range(QC):
            n_pairs = 2 * c + 2  # k-tile pairs (2p, 2p+1), p in [0, n_pairs)
            o_ps = [
                ps.tile([P, H + 1], F32, tag="o", bufs=4, name=f"o{c}_{s}")
                for s in range(4)
            ]
            st_ps = {}

            def emit_s_pair(p, c=c, st_ps=st_ps):
                st = ps.tile([P, 2 * CH], F32, tag="mm", bufs=2, name=f"st{c}_{p}")
                for h in range(2):
                    i = 2 * p + h
                    e0 = max(i - 4 * c, 0) * P
                    nc.tensor.matmul(
                        st[:, h * CH + e0 : (h + 1) * CH],
                        kT[:, i * P : (i + 1) * P],
                        qT[:, c * CH + e0 : (c + 1) * CH],
                        start=True,
                        stop=True,
                    )
                st_ps[p] = st

            emit_s_pair(0)
            for p in range(n_pairs):
                if p + 1 < n_pairs:
                    emit_s_pair(p + 1)  # keep PE fed while ACT does exp(p)
                st = st_ps.pop(p)
                pt = work.tile([P, 2 * CH], BF16, tag="pt", name=f"pt{c}_{p}")
                j0 = 2 * p - 4 * c  # diag offset of first tile in pair
                if j0 < 0:
                    # fully valid pair: one wide exp
                    nc.scalar.activation(
                        pt[:], st[:], mybir.ActivationFunctionType.Exp, scale=SCALE
                    )
                else:
                    for h in range(2):
                        e0 = (j0 + h) * P
                        nc.scalar.activation(
                            pt[:, h * CH + e0 : (h + 1) * CH],
                            st[:, h * CH + e0 : (h + 1) * CH],
                            mybir.ActivationFunctionType.Exp,
                            scale=SCALE,
                        )
                        # zero the causal triangle of the diagonal block
                        nc.gpsimd.affine_select(
                            out=pt[:, h * CH + e0 : h * CH + e0 + P],
                            in_=pt[:, h * CH + e0 : h * CH + e0 + P],
                            compare_op=mybir.AluOpType.is_ge,
                            fill=0.0,
                            base=0,
                            pattern=[[1, P]],
                            channel_multiplier=-1,
                        )
                for h in range(2):
                    i = 2 * p + h
                    for s in range(4):
                        if i <= 4 * c + s:
                            nc.tensor.matmul(
                                o_ps[s][:],
                                pt[:, h * CH + s * P : h * CH + (s + 1) * P],
                                v_aug[:, i, :],
                                start=(i == 0),
                                stop=(i == 4 * c + s),
                            )
            for s in range(4):
                qt_idx = 4 * c + s
                recip = work.tile([P, 1], F32, tag="recip", name=f"rcp{qt_idx}")
                nc.vector.reciprocal(recip[:], o_ps[s][:, H : H + 1])
                o_sb = work.tile([P, H], F32, tag="o_sb", name=f"o_sb{qt_idx}")
                nc.vector.tensor_scalar_mul(o_sb[:], o_ps[s][:, 0:H], recip[:])
                nc.sync.dma_start(out[qt_idx * P : (qt_idx + 1) * P, :], o_sb[:])


_NC_CACHE = None


def _get_nc():
    global _NC_CACHE
    if _NC_CACHE is None:
        _NC_CACHE = build_nc()
    return _NC_CACHE


def kernel(**inputs):
    x = np.ascontiguousarray(np.asarray(inputs["x"], dtype=np.float32))
    wq = np.ascontiguousarray(np.asarray(inputs["Wq"], dtype=np.float32))
    wk = np.ascontiguousarray(np.asarray(inputs["Wk"], dtype=np.float32))
    wv = np.ascontiguousarray(np.asarray(inputs["Wv"], dtype=np.float32))
    assert x.shape == (B, T, D)
    nc = _get_nc()
    in_maps = [
        {"x": np.ascontiguousarray(x[b]), "wq": wq, "wk": wk, "wv": wv}
        for b in range(N_CORES)
    ]
    res = bass_utils.run_bass_kernel_spmd(nc, in_maps, core_ids=list(range(N_CORES)))
    return np.stack([res.results[b]["out"] for b in range(N_CORES)], axis=0)

